# revision 53
# baseline (speedup 1.0000x reference)
"""Trainium2 Bass kernel v3 for nn_Attention_51238959841962.

GroupNorm(8) -> QKV 1x1 conv -> 8-head attention (n=1024, d=64) -> out
projection -> residual, x:[8,512,32,32]. Data-parallel over batch (8 cores).

v3 over v2 (v2: fp8 DoublePixel matmuls, exp shift -3.5, 65th-column softmax
denominator):
- unpacked q (K=64 partition-sliced sims), removing half the q PSUM drains;
- PSUM split 6/2 banks: three [128,1024] sim buffers keep both PSUM-capable
  exp engines (ACT true-exp + DVE fast-exp) saturated -- Pool/GPSIMD cannot
  access PSUM on TRN2, so all PSUM drains live on ACT+DVE and Pool takes the
  SBUF-only work (outc assembly, residual normalize) plus DMA issue;
- a static greedy balancer assigns every fungible elementwise op to the
  least-loaded legal engine (costs from the TRN2 cost model);
- software-pipelined iterations: pair-3 AV, reciprocal chain, scale and the
  out-projection of iteration r overlap the pair-0/1 sims of iteration r+1;
- reciprocal via direct SBUF->SBUF shuffle DMA of the denominator row into
  [128,16], one DRAM bounce for the unshuffle (partition-split DMA *sources*
  corrupt data, dest-side splits are fine; f32r-typed DMA also corrupts);
- bf16 outc / w_out (same PE cost as f32r, half the SBUF).
"""
import sys
sys.path.insert(0, "/opt/trn_rl_repo")
import numpy as np
import ml_dtypes
import concourse.bass as bass
import concourse.tile as tile
from concourse import mybir
from concourse.vector_clock import ScopedClock

# ---------------------------------------------------------------------------
# Walrus workaround: at most ONE sync-wait per engine instruction (see v1).
# ---------------------------------------------------------------------------
MAX_WAITS = 1


def _patched_drain(self, tick_clock, wait_clock):
    nc = self.nc
    probe = nc.sync.nop(nofuse=True, hint="drain_wait_split")
    wait_clock.add_sem_waits(probe.ins, ScopedClock({None: tick_clock.global_clock}))
    si = probe.ins.sync_info
    waits = list(si.on_wait or []) if si is not None else []
    if len(waits) > MAX_WAITS:
        si.on_wait = waits[:MAX_WAITS]
        rest = waits[MAX_WAITS:]
        for i in range(0, len(rest), MAX_WAITS):
            n2 = nc.sync.nop(nofuse=True, hint="drain_wait_split")
            n2.ins.sync_info = mybir.SyncInfo(on_wait=rest[i:i + MAX_WAITS], on_update=[])
    nc.sync.drain()
    nc.all_engine_barrier()
    popped = nc._tile_sem_poison_stack.pop()
    assert popped is self._sem_poison
    nc.clear_and_free_semaphores(list(self.sems.allocated().values()))
    nc.all_engine_barrier()


tile.TileContext._drain_and_barrier = _patched_drain

# ---------------------------------------------------------------------------
# Cost-model patch: CoreSim doesn't model DoublePixel fp8 matmuls (measured
# ~0.35x of the modeled bf16 cost on HW). Scale the exec delay so the tile
# scheduler sees realistic PE costs.
# ---------------------------------------------------------------------------
from concourse import cost_model as _cm
from concourse.cost_model_rust import Delay as _Delay

_orig_cm_visit = _cm.InstructionCostModel.visit
_FAST_MM = ("DoublePixel", "DoubleColumn")


def _patched_cm_visit(self, instruction, sim):
    tls = _orig_cm_visit(self, instruction, sim)
    pm = getattr(instruction, "perf_mode", None)
    if pm is not None and getattr(pm, "name", "") in _FAST_MM and \
            instruction.opcode == "Matmult":
        out = []
        for tl in tls:
            idx, best = None, 0.0
            for i, ev in enumerate(tl):
                if isinstance(ev, _Delay) and ev.ns > best:
                    idx, best = i, ev.ns
            if idx is not None:
                tl = list(tl)
                tl[idx] = _Delay(best * 0.35)
            out.append(tl)
        return out
    return tls


_cm.InstructionCostModel.visit = _patched_cm_visit


def split_waits(nc, max_waits=MAX_WAITS):
    for fn in nc.m.functions:
        for bb in fn.blocks:
            new_insts = []
            changed = False
            for inst in bb.instructions:
                si = getattr(inst, "sync_info", None)
                waits = list(si.on_wait) if (si is not None and si.on_wait) else []
                if len(waits) > max_waits:
                    extra = waits[:-max_waits]
                    si.on_wait = waits[-max_waits:]
                    for i in range(0, len(extra), max_waits):
                        nop = mybir.InstNoOp(name=f"waitsplit-{nc.next_id()}", ins=[], outs=[])
                        nop.engine = inst.engine
                        nop.sync_info = mybir.SyncInfo(on_wait=extra[i:i + max_waits], on_update=[])
                        new_insts.append(nop)
                    changed = True
                new_insts.append(inst)
            if changed:
                bb.instructions = new_insts
    return nc


# ---------------------------------------------------------------------------
C, NTOK, H, D, KT = 512, 1024, 8, 64, 4
EPS = 1e-5
SCALE = 0.125
CSH = 3.5                                   # exp(SCALE*sim - CSH)
FA = SCALE * np.log2(np.e) * 8.0            # fast-exp mult
FB = 56.0 - CSH * np.log2(np.e) * 8.0       # fast-exp add (7*8 bias - shift)

F32R = mybir.dt.float32r
F32 = mybir.dt.float32
BF16 = mybir.dt.bfloat16
F8 = mybir.dt.float8e4
U8 = mybir.dt.uint8
U16 = mybir.dt.uint16
AF = mybir.ActivationFunctionType
AL = mybir.AluOpType
DP = mybir.MatmulPerfMode.DoublePixel

# static greedy engine balancer: unit costs (ns) per op kind per engine
# a=ACT (scalar) d=DVE (vector) p=Pool (gpsimd)
# Pool (gpsimd/Q7) cannot access PSUM on TRN2, so every PSUM-reading
# kind is restricted to ACT+DVE; Pool gets SBUF-only work.
EW_COSTS = {
    "exp":   {"a": 1040, "d": 1190},
    "drain": {"a": 1040, "d": 1190},              # 1024-col PSUM->SBUF copy
    "vv":    {"a": 610,  "d": 660},               # 512-col PSUM->SBUF copy
    "tt":    {"d": 1190},                         # 1024-col tt w/ PSUM operand
    "tt5":   {"d": 660},                          # 512-col tt w/ PSUM operand
    "norm":  {"a": 1040, "d": 1130},              # x*a+b affine, critical path
    "normp": {"a": 1560, "d": 1700, "p": 1520},   # x*a+b affine, slack path
    "asm":   {"a": 1040, "d": 1130, "p": 1420},   # SBUF->SBUF copy 1024-col
}


def build_attn(nc, R=1, trace_sim=False, ep_bufs=49, abl=None):
    x_ext = nc.declare_dram_parameter("x", [C, NTOK], F32, isOutput=False)
    wqkv8_ext = nc.declare_dram_parameter("wqkv8", [C, 3 * C], U8, isOutput=False)
    woutT_ext = nc.declare_dram_parameter("woutT", [C, C], U16, isOutput=False)
    gb_ext = nc.declare_dram_parameter("gb", [128, 8], F32, isOutput=False)
    selw_ext = nc.declare_dram_parameter("selw", [128, 2], F32, isOutput=False)
    selT_ext = nc.declare_dram_parameter("selT", [2, 128], F32, isOutput=False)
    mapP_ext = nc.declare_dram_parameter("mapP", [2, 128], F32, isOutput=False)
    ident_ext = nc.declare_dram_parameter("ident", [128, 128], F32, isOutput=False)
    out_ext = nc.declare_dram_parameter("out", [C, NTOK], F32, isOutput=True)
    r_dram = [nc.dram_tensor(f"r_dram{p}", [2, NTOK], F32) for p in range(4)]

    # global greedy balancer state: accumulated ns per elementwise engine.
    # DVE pre-loaded for its exclusive work (bn_stats, reciprocal, smalls).
    ew_load = {"a": 300.0, "d": 6500.0 * R, "p": 5000.0 * R}

    def ew_pick(kind):
        menu = EW_COSTS[kind]
        k = min(menu, key=lambda k: ew_load[k] + menu[k])
        ew_load[k] += menu[k]
        return k

    with tile.TileContext(nc, trace_sim=trace_sim) as tc:
        with tc.tile_pool(name="wp", bufs=1) as wp, \
             tc.tile_pool(name="xp", bufs=2) as xp, \
             tc.tile_pool(name="xqp", bufs=2) as xqp, \
             tc.tile_pool(name="qkp", bufs=2) as qkp, \
             tc.tile_pool(name="vp", bufs=2) as vp, \
             tc.tile_pool(name="ep", bufs=ep_bufs) as ep, \
             tc.tile_pool(name="ocp", bufs=2) as ocp, \
             tc.tile_pool(name="smp", bufs=2) as smp, \
             tc.tile_pool(name="usp", bufs=7) as usp, \
             tc.tile_pool(name="ps_a", bufs=2, space="PSUM") as ps_a, \
             tc.tile_pool(name="ps_b", bufs=3, space="PSUM") as ps_b:

            # ---------------- persistent tiles (outside R loop) -----------
            gbt = wp.tile([128, 8], F32, tag="gb")
            nc.sync.dma_start(out=gbt, in_=gb_ext[:, :])
            selw_t = wp.tile([128, 2], F32R, tag="selw")
            nc.sync.dma_start(out=selw_t, in_=selw_ext[:, :].bitcast(F32R))
            selT_t = wp.tile([2, 128], F32R, tag="selT")
            nc.sync.dma_start(out=selT_t, in_=selT_ext[:, :].bitcast(F32R))
            mapP_t = wp.tile([2, 128], F32R, tag="mapP")
            nc.sync.dma_start(out=mapP_t, in_=mapP_ext[:, :].bitcast(F32R))
            ident = wp.tile([128, 128], F32R, tag="ident")
            nc.sync.dma_start(out=ident, in_=ident_ext[:, :].bitcast(F32R))
            epst = wp.tile([2, 1], F32, tag="eps")
            nc.vector.memset(epst, EPS)
            ebias = wp.tile([128, 1], F32, tag="ebias")
            nc.vector.memset(ebias, -CSH)

            w8 = []
            w_engines = [nc.sync, nc.gpsimd, nc.scalar, nc.gpsimd]
            for kt in range(KT):
                wt = wp.tile([128, 3 * C], U8, tag=f"w8_{kt}", name=f"w8_{kt}")
                w_engines[kt].dma_start(out=wt, in_=wqkv8_ext[kt*128:(kt+1)*128, :])
                w8.append(wt.bitcast(F8))
            woutT = []
            for kt in range(KT):
                w2 = wp.tile([128, C], U16, tag=f"wout{kt}", name=f"wout{kt}")
                w_engines[kt % 2].dma_start(out=w2, in_=woutT_ext[kt*128:(kt+1)*128, :])
                woutT.append(w2.bitcast(BF16))

            # q tiles [128, NTOK] f8: partitions 0:64 = head 2p, 64:128 = head
            # 2p+1 (PSUM layout); sims use K=64 partition slices, so no
            # zero-padding or block packing is needed.
            qp8_par = []
            vv8_par = []
            for par in range(2):
                qp8 = []
                for p in range(4):
                    q = wp.tile([128, NTOK], U8, tag=f"qp{p}_{par}", name=f"qp{p}_{par}")
                    qp8.append(q.bitcast(F8))
                qp8_par.append(qp8)
                vv8 = []
                for jt in range(8):
                    v = wp.tile([128, H * 68], U8, tag=f"vv{jt}_{par}", name=f"vv{jt}_{par}")
                    v8v = v.bitcast(F8).rearrange("p (h e) -> p h e", e=68)
                    nc.gpsimd.memset(v8v[:, :, 64:68], 0.0)
                    nc.gpsimd.memset(v8v[:, :, 64:65], 1.0)
                    vv8.append(v.bitcast(F8))
                vv8_par.append(vv8)

            ENG = {"d": nc.vector, "p": nc.gpsimd}

            def emit_norm(out_ap, in_ap, ab, kind="norm"):
                k = ew_pick(kind)
                if k == "a":
                    nc.scalar.activation(out=out_ap, in_=in_ap, func=AF.Identity,
                                         bias=ab[:, 1:2], scale=ab[:, 0:1])
                else:
                    ENG[k].tensor_scalar(out=out_ap, in0=in_ap,
                                         scalar1=ab[:, 0:1], scalar2=ab[:, 1:2],
                                         op0=AL.mult, op1=AL.add)

            def emit_drain(out_ap, in_ap, kind="drain"):
                k = ew_pick(kind)
                if k == "a":
                    nc.scalar.activation(out=out_ap, in_=in_ap, func=AF.Copy,
                                         scale=1.0)
                else:
                    ENG[k].tensor_copy(out=out_ap, in_=in_ap)

            def emit_tt(out_ap, in0, in1, op, kind="tt"):
                k = ew_pick(kind)
                ENG[k].tensor_tensor(out=out_ap, in0=in0, in1=in1, op=op)

            def load_x():
                x_eng = [nc.sync, nc.sync, nc.sync, nc.sync,
                         nc.sync, nc.sync, nc.sync, nc.sync]
                xt_new = []
                for kt in range(KT):
                    t = xp.tile([128, NTOK], F32R, tag=f"x{kt}", name=f"xt{kt}")
                    for h in range(2):
                        x_eng[2*kt + h].dma_start(
                            out=t[:, h*512:(h+1)*512],
                            in_=x_ext[kt*128:(kt+1)*128, h*512:(h+1)*512].bitcast(F32R))
                    xt_new.append(t)
                return xt_new

            def emit_prologue_kt(xt, xq8, kt):
                st = smp.tile([128, 2, 6], F32, tag="st")
                nc.vector.bn_stats(out=st[:, 0, :], in_=xt[kt][:, 0:512])
                nc.vector.bn_stats(out=st[:, 1, :], in_=xt[kt][:, 512:1024])
                mv = smp.tile([128, 2], F32, tag="mv")
                nc.vector.bn_aggr(out=mv, in_=st)
                t2 = smp.tile([128, 2], F32R, tag="t2")
                nc.vector.tensor_tensor(out=t2[:, 1:2], in0=mv[:, 0:1], in1=mv[:, 0:1], op=AL.mult)
                nc.vector.tensor_tensor(out=t2[:, 1:2], in0=t2[:, 1:2].bitcast(F32), in1=mv[:, 1:2], op=AL.add)
                nc.vector.tensor_copy(out=t2[:, 0:1], in_=mv[:, 0:1])
                gs_ps = ps_a.tile([2, 2], F32, tag="pa")
                nc.tensor.matmul(out=gs_ps, lhsT=selw_t, rhs=t2, start=True, stop=True)
                gs = smp.tile([2, 2], F32, tag="gs")
                nc.vector.tensor_copy(out=gs, in_=gs_ps)
                var2 = smp.tile([2, 1], F32, tag="var2")
                nc.vector.tensor_tensor(out=var2, in0=gs[:, 0:1], in1=gs[:, 0:1], op=AL.mult)
                nc.vector.tensor_tensor(out=var2, in0=gs[:, 1:2], in1=var2, op=AL.subtract)
                lnv = smp.tile([2, 1], F32, tag="lnv")
                nc.scalar.activation(out=lnv, in_=var2, func=AF.Ln, bias=epst, scale=1.0)
                gsr = smp.tile([2, 2], F32R, tag="gsr")
                nc.scalar.activation(out=gsr[:, 1:2], in_=lnv, func=AF.Exp, scale=-0.5)
                nc.vector.tensor_copy(out=gsr[:, 0:1], in_=gs[:, 0:1])
                bc_ps = ps_a.tile([128, 2], F32, tag="pa")
                nc.tensor.matmul(out=bc_ps, lhsT=selT_t, rhs=gsr, start=True, stop=True)
                ab = smp.tile([128, 2], F32, tag="ab", bufs=8)
                nc.vector.tensor_tensor(out=ab[:, 0:1], in0=bc_ps[:, 1:2], in1=gbt[:, 2*kt:2*kt+1], op=AL.mult)
                nc.vector.tensor_tensor(out=ab[:, 1:2], in0=bc_ps[:, 0:1], in1=ab[:, 0:1], op=AL.mult)
                nc.vector.tensor_tensor(out=ab[:, 1:2], in0=gbt[:, 2*kt+1:2*kt+2], in1=ab[:, 1:2], op=AL.subtract)
                xqt = xqp.tile([128, NTOK], U8, tag=f"xq{kt}", name=f"xq{kt}")
                emit_norm(xqt.bitcast(F8), xt[kt].bitcast(F32), ab)
                xq8[kt] = xqt.bitcast(F8)
                emit_norm(xt[kt], xt[kt].bitcast(F32), ab, kind="normp")

            def emit_prologue():
                xt = load_x()
                xq8 = [None] * KT
                return xt, xq8

            cur_state = emit_prologue()
            for kt in range(KT):
                emit_prologue_kt(cur_state[0], cur_state[1], kt)

            prev_chunks = []
            for _r in range(R):
                xt, xq8 = cur_state
                qp8 = qp8_par[_r % 2]
                vv8 = vv8_par[_r % 2]
                nxt_state = emit_prologue() if _r + 1 < R else None

                kp8 = {}
                ei = [0]
                Es = {}
                outc = {}
                srss = {}
                usbs = {}
                rts = {}

                def emit_qkproj(p):
                    # q: out channels p*128..(p+1)*128 ; k: 512 + p*128 ...
                    # one 1-bank PSUM tile + drain per 512-col iN block
                    for qk in range(2):
                        off = qk * C   # 0 for q, C for k
                        if qk == 0:
                            dst = qp8[p]
                        else:
                            kt8 = qkp.tile([128, NTOK], U8, tag=f"k{p}", name=f"k{p}")
                            kp8[p] = dst = kt8.bitcast(F8)
                        for iN in range(2):
                            ps = ps_a.tile([128, 512], F32, tag="pa",
                                           name=f"qk{p}_{qk}_{iN}")
                            for kt in range(KT):
                                nc.tensor.matmul(out=ps,
                                                 lhsT=w8[kt][:, off + p*128:off + (p+1)*128],
                                                 rhs=xq8[kt][:, iN*512:(iN+1)*512],
                                                 start=(kt == 0), stop=(kt == KT-1),
                                                 perf_mode=DP)
                            emit_drain(dst[:, iN*512:(iN+1)*512], ps, kind="vv")

                def emit_vproj(jt):
                    vps = ps_a.tile([128, 512], F32, tag="pa", name=f"vps{jt}")
                    for kt in range(KT):
                        nc.tensor.matmul(out=vps,
                                         lhsT=xq8[kt][:, jt*128:(jt+1)*128],
                                         rhs=w8[kt][:, 2*C:3*C],
                                         start=(kt == 0), stop=(kt == KT-1),
                                         perf_mode=DP)
                    emit_drain(vv8[jt].rearrange("p (h e) -> p h e", e=68)[:, :, 0:64],
                               vps.rearrange("p (h e) -> p h e", e=64), kind="vv")

                def emit_sim(p, hh, jt):
                    # hh in {0,1}: head 2p+hh ; K=64 partition slice of k/q
                    pss = ps_b.tile([128, NTOK], F32, tag="pb", name=f"sim{p}_{hh}_{jt}")
                    for iN in range(1 if abl == "halfsim" else 2):
                        nc.tensor.matmul(out=pss[:, iN*512:(iN+1)*512],
                                         lhsT=kp8[p][hh*64:(hh+1)*64, jt*128:(jt+1)*128],
                                         rhs=qp8[p][hh*64:(hh+1)*64, iN*512:(iN+1)*512],
                                         start=True, stop=True, perf_mode=DP)
                    et = ep.tile([128, NTOK], U8, tag="e", name=f"e{p}_{hh}_{jt}")
                    esl = slice(0, 512) if abl == "halfexp" else slice(0, NTOK)
                    # strict alternation keeps consecutive sim buffers draining
                    # on different engines; extra ops go to the cheaper ACT
                    k = "adaadad"[ei[0] % 7]
                    ei[0] += 1
                    ew_load[k] += EW_COSTS["exp"][k]
                    if k == "a":
                        nc.scalar.activation(out=et.bitcast(F8)[:, esl], in_=pss[:, esl],
                                             func=AF.Exp, bias=ebias, scale=SCALE)
                    else:
                        ENG[k].tensor_scalar(out=et[:, esl], in0=pss[:, esl],
                                             scalar1=float(FA),
                                             scalar2=float(FB), op0=AL.mult, op1=AL.add)
                    Es[(p, hh, jt)] = et.bitcast(F8)

                def emit_av(p, hh, srss=srss, usbs=usbs, Es=Es, vv8=vv8):
                    h = 2 * p + hh
                    if hh == 0:
                        srss[p] = smp.tile([128, 16], F32, tag="srs", bufs=4,
                                           name=f"srs{p}")
                    usb = usp.tile([65, NTOK], F32, tag="u", name=f"usb{p}_{hh}")
                    njt = 4 if abl == "halfav" else 8
                    for iN in range(2):
                        ups = ps_a.tile([65, 512], F32, tag="pa",
                                        name=f"ups{p}_{hh}_{iN}")
                        for jt in range(njt):
                            nc.tensor.matmul(out=ups,
                                             lhsT=vv8[jt].rearrange("p (h e) -> p h e", e=68)[:, h, 0:65],
                                             rhs=Es[(p, hh, jt)][:, iN*512:(iN+1)*512],
                                             start=(jt == 0), stop=(jt == njt - 1),
                                             perf_mode=DP)
                        emit_drain(usb[:, iN*512:(iN+1)*512], ups, kind="vv")
                    usbs[(p, hh)] = usb
                    # denominator row straight into the partition-shuffled
                    # reciprocal layout (SBUF->SBUF dma, no DRAM round trip);
                    # one dma per iN half so each read is scoped to exactly
                    # the drain that produced it
                    for iN in range(2):
                        nc.sync.dma_start(
                            out=srss[p][hh*64 + iN*32:hh*64 + (iN+1)*32, :],
                            in_=usb[64:65, iN*512:(iN+1)*512].rearrange(
                                "o (q f) -> o q f", f=16))

                def emit_r_chain(p, srss=srss, rts=rts):
                    # partition-split SOURCE APs are not DMA-safe, so the
                    # unshuffle bounces through DRAM (flat on the DRAM side)
                    srs = srss[p]
                    nc.vector.reciprocal(out=srs, in_=srs)
                    rt = smp.tile([2, NTOK], F32, tag="rr", bufs=4, name=f"rt{p}")
                    rts[p] = rt
                    rb = r_dram[p]
                    nc.gpsimd.dma_start(
                        out=rb.ap().rearrange("a (q f) -> (a q) f", f=16),
                        in_=srs)
                    nc.sync.dma_start(out=rt, in_=rb[:, :])

                def emit_asm(p, outc=outc, usbs=usbs):
                    # outc assembly (SBUF->SBUF, Pool-eligible): usb halves
                    # into the pair-contiguous outc tile; no rt dependency
                    outc[p] = ocp.tile([128, NTOK], BF16, tag=f"oc{p}", name=f"oc{p}")
                    for hh in range(2):
                        emit_drain(outc[p][hh*64:(hh+1)*64, :],
                                   usbs[(p, hh)][0:64, :], kind="asm")

                def emit_scale(p, rts=rts, outc=outc):
                    # in-place r-scale of outc against the PSUM broadcast
                    rt = rts[p]
                    for iN in range(2):
                        rps = ps_a.tile([128, 512], F32, tag="pa", name=f"rps{p}_{iN}")
                        nc.tensor.matmul(out=rps, lhsT=mapP_t,
                                         rhs=rt.bitcast(F32R)[:, iN*512:(iN+1)*512],
                                         start=True, stop=True)
                        emit_tt(outc[p][:, iN*512:(iN+1)*512],
                                outc[p][:, iN*512:(iN+1)*512],
                                rps, AL.mult, kind="tt5")

                def emit_outproj_ot(ot, outc=outc, xt=xt):
                    # out projection for channel block ot: two 1-bank groups
                    out_eng = [nc.sync, nc.sync, nc.sync, nc.sync]
                    for iN in range(2):
                        pps = ps_a.tile([128, 512], F32, tag="pa",
                                        name=f"pps{ot}_{iN}")
                        for kt in range(KT):
                            nc.tensor.matmul(out=pps,
                                             lhsT=woutT[kt][:, ot*128:(ot+1)*128],
                                             rhs=outc[kt][:, iN*512:(iN+1)*512],
                                             start=(kt == 0), stop=(kt == KT-1))
                        emit_tt(xt[ot][:, iN*512:(iN+1)*512], pps,
                                xt[ot].bitcast(F32)[:, iN*512:(iN+1)*512],
                                AL.add, kind="tt5")
                    out_eng[ot].dma_start(out=out_ext[ot*128:(ot+1)*128, :],
                                          in_=xt[ot].bitcast(F32))

                # ---------------- emission schedule ----------------
                # software-pipelined: the previous iteration's tail (AV of
                # pair 3, r-chain, scales, out projection) interleaves with
                # this iteration's pair-0 sims so exp engines never drain.
                emit_qkproj(0)
                tail = list(prev_chunks)
                prev_chunks = []
                if tail:
                    # av'(3,0/1), asm'(3); qkproj(1) before the prev r-chain
                    # and out-projection so pair-1 sims aren't starved of PSUM
                    chunks = tail[0:3]
                    chunks.append(lambda: emit_qkproj(1))
                    chunks.extend(tail[3:6])           # rc'(3), sc'(2), sc'(3)
                    carry = tail[6:]                   # out-projection x4
                else:
                    chunks = [lambda: emit_qkproj(1)]
                    carry = []
                chunks.extend(lambda jt=jt: emit_vproj(jt) for jt in range(4))
                chunks.extend(carry[0:2])
                ci = 0
                for hh in range(2):
                    for jt in range(8):
                        emit_sim(0, hh, jt)
                        if ci < len(chunks):
                            chunks[ci]()
                            ci += 1
                for c in chunks[ci:]:
                    c()

                for p in range(1, 4):
                    prev = p - 1
                    chunks = []
                    if p == 1:
                        chunks.extend(carry[2:])
                        chunks.extend(lambda jt=jt: emit_vproj(jt) for jt in range(4, 8))
                    if p < 3:
                        chunks.append(lambda o=p+1: emit_qkproj(o))
                    chunks.append(lambda q=prev: emit_av(q, 0))
                    chunks.append(lambda q=prev: emit_av(q, 1))
                    chunks.append(lambda q=prev: emit_asm(q))
                    chunks.append(lambda q=prev: emit_r_chain(q))
                    if prev >= 1:
                        chunks.append(lambda q=prev-1: emit_scale(q))
                    if p >= 2 and nxt_state is not None:
                        for kt in ((0, 1) if p == 2 else (2, 3)):
                            chunks.append(lambda k=kt: emit_prologue_kt(
                                nxt_state[0], nxt_state[1], k))
                    cadence = 1 if p == 1 else 2
                    ci = 0
                    for hh in range(2):
                        for jt in range(8):
                            emit_sim(p, hh, jt)
                            if ci < len(chunks) and jt % cadence == cadence - 1:
                                chunks[ci]()
                                ci += 1
                    for c in chunks[ci:]:
                        c()

                prev_chunks = [
                    lambda f=emit_av: f(3, 0),
                    lambda f=emit_av: f(3, 1),
                    lambda f=emit_asm: f(3),
                    lambda f=emit_r_chain: f(3),
                    lambda f=emit_scale: f(2),
                    lambda f=emit_scale: f(3),
                ]
                prev_chunks.extend(lambda ot=ot, f=emit_outproj_ot: f(ot)
                                   for ot in range(4))
                cur_state = nxt_state

            # final iteration's tail has no successor to hide behind
            for c in prev_chunks:
                c()
    return nc


def host_inputs(x_b, gamma, beta, w_qkv, w_out):
    gb = np.zeros((128, 8), np.float32)
    for kt in range(KT):
        gb[:, 2*kt] = gamma[kt*128:(kt+1)*128]
        gb[:, 2*kt+1] = beta[kt*128:(kt+1)*128]
    selw = np.zeros((128, 2), np.float32)
    selw[0:64, 0] = 1.0 / 64
    selw[64:128, 1] = 1.0 / 64
    selT = np.zeros((2, 128), np.float32)
    selT[0, 0:64] = 1.0
    selT[1, 64:128] = 1.0
    mapP = np.zeros((2, 128), np.float32)
    mapP[0, 0:64] = 1.0
    mapP[1, 64:128] = 1.0
    w8 = np.asarray(w_qkv.T, dtype=ml_dtypes.float8_e4m3)  # [C_in, 3C_out]
    return {
        "x": np.ascontiguousarray(x_b.reshape(C, NTOK)),
        "wqkv8": np.ascontiguousarray(w8.view(np.uint8)),
        "woutT": np.ascontiguousarray(
            w_out.T.astype(ml_dtypes.bfloat16)).view(np.uint16),
        "gb": gb, "selw": selw, "selT": selT, "mapP": mapP,
        "ident": np.eye(128, dtype=np.float32),
    }


# ---------------------------------------------------------------------------
_CACHE = {}


def _get_runner():
    if "run" in _CACHE:
        return _CACHE["run"]
    import jax
    from jax.sharding import Mesh, PartitionSpec, NamedSharding
    from jax.experimental.shard_map import shard_map
    from concourse import bass2jax

    bass2jax.install_neuronx_cc_hook()
    nc = bass.Bass()
    build_attn(nc)
    split_waits(nc)

    partition_name = nc.partition_id_tensor.name if nc.partition_id_tensor else None
    in_names, out_names, out_avals = [], [], []
    for alloc in nc.m.functions[0].allocations:
        if not isinstance(alloc, mybir.MemoryLocationSet):
            continue
        name = alloc.memorylocations[0].name
        if alloc.kind == "ExternalInput":
            if name != partition_name:
                in_names.append(name)
        elif alloc.kind == "ExternalOutput":
            out_names.append(name)
            out_avals.append(jax.core.ShapedArray(tuple(alloc.tensor_shape),
                                                  mybir.dt.np(alloc.dtype)))
    n_params = len(in_names)
    all_in_names = in_names + out_names
    if partition_name is not None:
        all_in_names.append(partition_name)

    def _body(*args):
        operands = list(args)
        if partition_name is not None:
            operands.append(bass2jax.partition_id_tensor())
        outs = bass2jax._bass_exec_p.bind(
            *operands, out_avals=tuple(out_avals), in_names=tuple(all_in_names),
            out_names=tuple(out_names), lowering_input_output_aliases=(),
            sim_require_finite=True, sim_require_nnan=True, nc=nc)
        return tuple(outs)

    n_cores = 8
    devices = jax.devices()[:n_cores]
    mesh = Mesh(np.asarray(devices), ("core",))
    in_specs = (PartitionSpec("core"),) * (n_params + len(out_avals))
    out_specs = (PartitionSpec("core"),) * len(out_avals)
    sharded = jax.jit(
        shard_map(_body, mesh=mesh, in_specs=in_specs, out_specs=out_specs,
                  check_rep=False),
        keep_unused=True)

    def run(in_maps):
        import jax as _jax
        per_core = [[np.asarray(m[name]) for name in in_names] for m in in_maps]
        concat_in = [np.concatenate([per_core[c][i] for c in range(n_cores)], axis=0)
                     for i in range(n_params)]
        concat_zeros = [np.zeros((n_cores * a.shape[0], *a.shape[1:]), a.dtype)
                        for a in out_avals]
        out_arrs = _jax.block_until_ready(sharded(*concat_in, *concat_zeros))
        return [
            {name: np.asarray(out_arrs[i]).reshape(n_cores, *out_avals[i].shape)[c]
             for i, name in enumerate(out_names)}
            for c in range(n_cores)
        ]

    _CACHE["run"] = run
    return run


def kernel(x, gamma, beta, w_qkv, w_out, b_out):
    x = np.asarray(x, dtype=np.float32)
    gamma = np.asarray(gamma, dtype=np.float32)
    beta = np.asarray(beta, dtype=np.float32)
    w_qkv = np.asarray(w_qkv, dtype=np.float32)
    w_out = np.asarray(w_out, dtype=np.float32)
    b_out = np.asarray(b_out, dtype=np.float32)
    b, c, h, w = x.shape
    assert (b, c, h, w) == (8, C, 32, 32)

    run = _get_runner()
    in_maps = [host_inputs(x[i], gamma, beta, w_qkv, w_out) for i in range(b)]
    last_err = None
    for _attempt in range(4):
        try:
            res = run(in_maps)
            break
        except Exception as e:
            last_err = e
            import time as _t
            _t.sleep(2.0)
    else:
        raise last_err
    out = np.stack([res[i]["out"].reshape(c, h, w) for i in range(b)])
    out = out + b_out[None, :, None, None]
    return out.astype(np.float32)

